# revision 5
# baseline (speedup 1.0000x reference)
"""ColumnParallelLinear kernel for Trainium2 (8 NeuronCores).

Computes Y[s,b,o] = sum_h X[s,b,h] * W[o,h]  (F.linear / einsum 'sbh,oh->sbo')
with S,B,H,OUT = 2048,4,1024,4096, fp32.

Strategy:
  - Flatten tokens: M = S*B = 8192 rows.  GEMM: [M,H] @ [H,OUT].
  - 2D shard over 8 cores: 4 token groups (2048 rows) x 2 out-column
    groups (2048 cols).  This minimizes per-core HBM traffic
    (x 8.4MB + w 8.4MB + y 16.8MB = 33.6MB/core) vs pure column- or
    row-parallel, keeping the kernel compute-bound.
  - Host pre-transposes X and W so the contraction dim (h) is
    outermost; on-chip tiles then have h on SBUF partitions with fully
    contiguous DMA, no on-device transposes.
  - Matmuls run as float32r (fp32 bits, full-rate PE path; moving dim
    512), accumulating fp32 in PSUM.
"""

import numpy as np

import concourse.bass as bass
from concourse import bacc
import concourse.mybir as mybir
import concourse.tile as tile
from concourse.bass_utils import run_bass_kernel_spmd

S, B, H, OUT = 2048, 4, 1024, 4096
M = S * B

N_CORES = 8
G_ROW, G_COL = 4, 2          # token groups x out-feature groups
M_LOC = M // G_ROW           # 2048 rows per core
N_LOC = OUT // G_COL         # 2048 out features per core

P = 128
KO = H // P                  # 8 contraction subtiles
MO = M_LOC // P              # 16 row tiles
NT = 512                     # psum free dim (one bank of fp32)
NO = N_LOC // NT             # 4 col tiles

MM_DT = mybir.dt.float32r    # full-rate fp32 matmul path


def build_nc(mm_dt=MM_DT):
    nc = bacc.Bacc(None, target_bir_lowering=False)
    xT = nc.declare_dram_parameter("xT", [H, M_LOC], mybir.dt.float32, isOutput=False)
    wT = nc.declare_dram_parameter("wT", [H, N_LOC], mybir.dt.float32, isOutput=False)
    y = nc.declare_dram_parameter("y", [M_LOC, N_LOC], mybir.dt.float32, isOutput=True)

    # h on partitions: row h = ko*P + p
    xT_r = xT[:, :].rearrange("(ko p) m -> p ko m", p=P)
    wT_r = wT[:, :].rearrange("(ko p) n -> p ko n", p=P)
    y_r = y[:, :].rearrange("(mo p) n -> p mo n", p=P)

    XG = 512                       # x mo-group width (4 row tiles)
    NXG = M_LOC // XG              # 4
    # DMA issue order: w0, x0, w1, x1, ... — and block (n, g) processing
    # order chosen so each block's inputs have arrived by the time the PE
    # reaches it (arrival-matched schedule; DMA rate is the startup limit).
    BLOCK_ORDER = [
        (0, 0), (1, 0), (0, 1), (1, 1),
        (2, 0), (2, 1), (0, 2), (1, 2), (2, 2),
        (3, 0), (3, 1), (3, 2), (0, 3), (1, 3), (2, 3), (3, 3),
    ]
    assert sorted(BLOCK_ORDER) == sorted((n, g) for n in range(NO) for g in range(NXG))

    with tile.TileContext(nc) as tc:
        with (
            tc.tile_pool(name="xp", bufs=1) as xp,
            tc.tile_pool(name="wp", bufs=1) as wp,
            tc.tile_pool(name="op", bufs=4) as op,
            tc.tile_pool(name="psp", bufs=8, space="PSUM") as psp,
        ):
            # all of x and w stay resident in SBUF (~17MB)
            x_sb = [[None] * KO for _ in range(NXG)]
            w_sb = [[None] * KO for _ in range(NO)]
            for i in range(max(NXG, NO)):
                if i < NO:
                    for k in range(KO):
                        wk = wp.tile([P, NT], mm_dt, tag=f"w{i}_{k}")
                        nc.sync.dma_start(
                            wk[:], wT_r[:, k, i * NT:(i + 1) * NT].bitcast(mm_dt)
                        )
                        w_sb[i][k] = wk
                if i < NXG:
                    for k in range(KO):
                        xk = xp.tile([P, XG], mm_dt, tag=f"x{i}_{k}")
                        nc.sync.dma_start(
                            xk[:], xT_r[:, k, i * XG:(i + 1) * XG].bitcast(mm_dt)
                        )
                        x_sb[i][k] = xk

            for n, g in BLOCK_ORDER:
                for mi in range(XG // P):
                    mo = g * (XG // P) + mi
                    ps = psp.tile([P, NT], mybir.dt.float32)
                    for k in range(KO):
                        nc.tensor.matmul(
                            ps[:],
                            lhsT=x_sb[g][k][:, mi * P:(mi + 1) * P],
                            rhs=w_sb[n][k][:],
                            start=(k == 0),
                            stop=(k == KO - 1),
                        )
                    o_sb = op.tile([P, NT], mybir.dt.float32)
                    nc.vector.tensor_copy(o_sb[:], ps[:])
                    # outputs ride the ACT HWDGE queue group so they don't
                    # contend with input loads on the SP queues
                    nc.scalar.dma_start(y_r[:, mo, n * NT:(n + 1) * NT], o_sb[:])
    nc.compile()
    return nc


def make_in_maps(input_, weight):
    X = np.ascontiguousarray(np.asarray(input_, dtype=np.float32).reshape(M, H))
    XT = np.ascontiguousarray(X.T)                                   # [H, M]
    WT = np.ascontiguousarray(np.asarray(weight, dtype=np.float32).T)  # [H, OUT]
    in_maps = []
    for c in range(N_CORES):
        i, j = divmod(c, G_COL)
        in_maps.append({
            "xT": np.ascontiguousarray(XT[:, i * M_LOC:(i + 1) * M_LOC]),
            "wT": np.ascontiguousarray(WT[:, j * N_LOC:(j + 1) * N_LOC]),
        })
    return in_maps


def assemble(results):
    Y = np.empty((M, OUT), dtype=np.float32)
    for c in range(N_CORES):
        i, j = divmod(c, G_COL)
        Y[i * M_LOC:(i + 1) * M_LOC, j * N_LOC:(j + 1) * N_LOC] = results[c]["y"]
    return Y.reshape(S, B, OUT)


def kernel(input_, weight):
    nc = build_nc()
    res = run_bass_kernel_spmd(nc, make_in_maps(input_, weight), list(range(N_CORES)))
    return assemble(res.results)


# revision 6
# speedup vs baseline: 1.0469x; 1.0469x over previous
"""ColumnParallelLinear kernel for Trainium2 (8 NeuronCores).

Computes Y[s,b,o] = sum_h X[s,b,h] * W[o,h]  (F.linear / einsum 'sbh,oh->sbo')
with S,B,H,OUT = 2048,4,1024,4096, fp32.

Strategy:
  - Flatten tokens: M = S*B = 8192 rows.  GEMM: [M,H] @ [H,OUT].
  - 2D shard over 8 cores: 4 token groups (2048 rows) x 2 out-column
    groups (2048 cols).  This minimizes per-core HBM traffic
    (x 8.4MB + w 8.4MB + y 16.8MB = 33.6MB/core) vs pure column- or
    row-parallel, keeping the kernel compute-bound.
  - Host pre-transposes X and W so the contraction dim (h) is
    outermost; on-chip tiles then have h on SBUF partitions with fully
    contiguous DMA, no on-device transposes.
  - Matmuls run as float32r (fp32 bits, full-rate PE path; moving dim
    512), accumulating fp32 in PSUM.
"""

import numpy as np

import concourse.bass as bass
from concourse import bacc
import concourse.mybir as mybir
import concourse.tile as tile
from concourse.bass_utils import run_bass_kernel_spmd

S, B, H, OUT = 2048, 4, 1024, 4096
M = S * B

N_CORES = 8
G_ROW, G_COL = 4, 2          # token groups x out-feature groups
M_LOC = M // G_ROW           # 2048 rows per core
N_LOC = OUT // G_COL         # 2048 out features per core

P = 128
KO = H // P                  # 8 contraction subtiles
MO = M_LOC // P              # 16 row tiles
NT = 512                     # psum free dim (one bank of fp32)
NO = N_LOC // NT             # 4 col tiles

MM_DT = mybir.dt.float32r    # full-rate fp32 matmul path


def build_nc(mm_dt=MM_DT):
    nc = bacc.Bacc(None, target_bir_lowering=False)
    xT = nc.declare_dram_parameter("xT", [H, M_LOC], mybir.dt.float32, isOutput=False)
    wT = nc.declare_dram_parameter("wT", [H, N_LOC], mybir.dt.float32, isOutput=False)
    y = nc.declare_dram_parameter("y", [M_LOC, N_LOC], mybir.dt.float32, isOutput=True)

    # h on partitions: row h = ko*P + p
    xT_r = xT[:, :].rearrange("(ko p) m -> p ko m", p=P)
    wT_r = wT[:, :].rearrange("(ko p) n -> p ko n", p=P)
    y_r = y[:, :].rearrange("(mo p) n -> p mo n", p=P)

    XG = 512                       # x mo-group width (4 row tiles)
    NXG = M_LOC // XG              # 4
    WPW = 1024                     # w chunk width (2 n tiles) -> 4KB dma runs
    NPAIR = N_LOC // WPW           # 2
    # Processing blocks (npair, g) ordered so each block's inputs have
    # arrived by the time the PE reaches it (DMA rate limits the startup).
    # DMA issue order: w01, x0, x1, w23, x2, x3.
    BLOCK_ORDER = [(0, 0), (0, 1), (1, 0), (0, 2), (1, 1), (0, 3), (1, 2), (1, 3)]
    assert sorted(BLOCK_ORDER) == sorted(
        (pi, g) for pi in range(NPAIR) for g in range(NXG)
    )

    with tile.TileContext(nc) as tc:
        with (
            tc.tile_pool(name="xp", bufs=1) as xp,
            tc.tile_pool(name="wp", bufs=1) as wp,
            tc.tile_pool(name="op", bufs=6) as op,
            tc.tile_pool(name="psp", bufs=8, space="PSUM") as psp,
        ):
            # all of x and w stay resident in SBUF (~17MB)
            x_sb = [[None] * KO for _ in range(NXG)]
            w_sb = [[None] * KO for _ in range(NPAIR)]

            def load_x(g):
                for k in range(KO):
                    xk = xp.tile([P, XG], mm_dt, tag=f"x{g}_{k}")
                    nc.sync.dma_start(
                        xk[:], xT_r[:, k, g * XG:(g + 1) * XG].bitcast(mm_dt)
                    )
                    x_sb[g][k] = xk

            def load_w(pi):
                for k in range(KO):
                    wk = wp.tile([P, WPW], mm_dt, tag=f"w{pi}_{k}")
                    nc.sync.dma_start(
                        wk[:], wT_r[:, k, pi * WPW:(pi + 1) * WPW].bitcast(mm_dt)
                    )
                    w_sb[pi][k] = wk

            load_w(0)
            load_x(0)
            load_x(1)
            load_w(1)
            load_x(2)
            load_x(3)

            for pi, g in BLOCK_ORDER:
                for mi in range(XG // P):
                    mo = g * (XG // P) + mi
                    stage = op.tile([P, WPW], mybir.dt.float32)
                    for li in range(WPW // NT):
                        ps = psp.tile([P, NT], mybir.dt.float32)
                        for k in range(KO):
                            nc.tensor.matmul(
                                ps[:],
                                lhsT=x_sb[g][k][:, mi * P:(mi + 1) * P],
                                rhs=w_sb[pi][k][:, li * NT:(li + 1) * NT],
                                start=(k == 0),
                                stop=(k == KO - 1),
                            )
                        nc.vector.tensor_copy(stage[:, li * NT:(li + 1) * NT], ps[:])
                    # full row-half per mo -> 4KB write runs; ACT HWDGE queue
                    # group so writes don't contend with input loads on SP
                    nc.scalar.dma_start(
                        y_r[:, mo, pi * WPW:(pi + 1) * WPW], stage[:]
                    )
    nc.compile()
    return nc


def make_in_maps(input_, weight):
    X = np.ascontiguousarray(np.asarray(input_, dtype=np.float32).reshape(M, H))
    XT = np.ascontiguousarray(X.T)                                   # [H, M]
    WT = np.ascontiguousarray(np.asarray(weight, dtype=np.float32).T)  # [H, OUT]
    in_maps = []
    for c in range(N_CORES):
        i, j = divmod(c, G_COL)
        in_maps.append({
            "xT": np.ascontiguousarray(XT[:, i * M_LOC:(i + 1) * M_LOC]),
            "wT": np.ascontiguousarray(WT[:, j * N_LOC:(j + 1) * N_LOC]),
        })
    return in_maps


def assemble(results):
    Y = np.empty((M, OUT), dtype=np.float32)
    for c in range(N_CORES):
        i, j = divmod(c, G_COL)
        Y[i * M_LOC:(i + 1) * M_LOC, j * N_LOC:(j + 1) * N_LOC] = results[c]["y"]
    return Y.reshape(S, B, OUT)


def kernel(input_, weight):
    nc = build_nc()
    res = run_bass_kernel_spmd(nc, make_in_maps(input_, weight), list(range(N_CORES)))
    return assemble(res.results)


# revision 9
# speedup vs baseline: 1.1126x; 1.0628x over previous
"""ColumnParallelLinear kernel for Trainium2 (8 NeuronCores).

Computes Y[s,b,o] = sum_h X[s,b,h] * W[o,h]  (F.linear / einsum 'sbh,oh->sbo')
with S,B,H,OUT = 2048,4,1024,4096, fp32.

Strategy:
  - Flatten tokens: M = S*B = 8192 rows.  GEMM: [M,H] @ [H,OUT].
  - 2D shard over 8 cores: 4 token groups (2048 rows) x 2 out-column
    groups (2048 cols).  This minimizes per-core HBM traffic
    (x 8.4MB + w 8.4MB + y 16.8MB = 33.6MB/core) vs pure column- or
    row-parallel, keeping the kernel compute-bound.
  - Host packs X and W into [chunk][partition][k][free] layout so every
    DMA descriptor moves a 16KB contiguous run (DMA is descriptor-rate
    limited at small runs); y is written as full 8KB rows.
  - Matmuls run as float32r (fp32 bits, full-rate PE path; moving dim
    512), accumulating fp32 in PSUM.
"""

import numpy as np

import concourse.bass as bass
from concourse import bacc
import concourse.mybir as mybir
import concourse.tile as tile
from concourse.bass_utils import run_bass_kernel_spmd

S, B, H, OUT = 2048, 4, 1024, 4096
M = S * B

N_CORES = 8
G_ROW, G_COL = 4, 2          # token groups x out-feature groups
M_LOC = M // G_ROW           # 2048 rows per core
N_LOC = OUT // G_COL         # 2048 out features per core

P = 128
KO = H // P                  # 8 contraction subtiles
NT = 512                     # psum free dim (one fp32 bank)
NO = N_LOC // NT             # 4 col tiles
XG = 512                     # x chunk width (4 row tiles)
NXG = M_LOC // XG            # 4 chunks
MO = M_LOC // P              # 16 row tiles

MM_DT = mybir.dt.float32r    # full-rate fp32 matmul path


def build_nc(mm_dt=MM_DT):
    nc = bacc.Bacc(None, target_bir_lowering=False)
    # packed inputs: [chunk][partition p][k][free] so each partition's slice
    # of one chunk is 16KB contiguous in DRAM (one descriptor per partition)
    xH = nc.declare_dram_parameter("xH", [NXG, P, KO, XG], mybir.dt.float32,
                                   isOutput=False)
    wH = nc.declare_dram_parameter("wH", [NO, P, KO, NT], mybir.dt.float32,
                                   isOutput=False)
    y = nc.declare_dram_parameter("y", [M_LOC, N_LOC], mybir.dt.float32,
                                  isOutput=True)
    y_r = y[:, :].rearrange("(mo p) n -> p mo n", p=P)

    with tile.TileContext(nc) as tc:
        with (
            tc.tile_pool(name="xp", bufs=1) as xp,
            tc.tile_pool(name="wp", bufs=1) as wp,
            tc.tile_pool(name="op", bufs=2) as op,
            tc.tile_pool(name="psp", bufs=8, space="PSUM") as psp,
        ):
            x_sb = [None] * NXG
            w_sb = [None] * NO

            def load_x(g):
                x_sb[g] = xp.tile([P, KO, XG], mm_dt, tag=f"x{g}", name=f"x{g}")
                nc.sync.dma_start(x_sb[g][:], xH[g, :, :, :].bitcast(mm_dt))

            def load_w(n):
                w_sb[n] = wp.tile([P, KO, NT], mm_dt, tag=f"w{n}", name=f"w{n}")
                nc.sync.dma_start(w_sb[n][:], wH[n, :, :, :].bitcast(mm_dt))

            # arrival order matches consumption: g0 runs n-outer so it only
            # needs w_n just-in-time while the rest streams in
            load_w(0)
            load_x(0)
            load_w(1)
            load_w(2)
            load_w(3)
            load_x(1)
            load_x(2)
            load_x(3)

            def do_group(g, n_outer):
                stages = [op.tile([P, N_LOC], mybir.dt.float32, tag=f"st{mi}",
                                  name=f"st{g}_{mi}")
                          for mi in range(XG // P)]
                outer = range(NO) if n_outer else range(XG // P)
                inner = range(XG // P) if n_outer else range(NO)
                for a in outer:
                    for b in inner:
                        n, mi = (a, b) if n_outer else (b, a)
                        ps = psp.tile([P, NT], mybir.dt.float32)
                        for k in range(KO):
                            nc.tensor.matmul(
                                ps[:],
                                lhsT=x_sb[g][:, k, mi * P:(mi + 1) * P],
                                rhs=w_sb[n][:, k, :],
                                start=(k == 0),
                                stop=(k == KO - 1),
                            )
                        nc.vector.tensor_copy(
                            stages[mi][:, n * NT:(n + 1) * NT], ps[:]
                        )
                # full 8KB-run row writes on the ACT HWDGE ring
                for mi in range(XG // P):
                    mo = g * (XG // P) + mi
                    nc.scalar.dma_start(y_r[:, mo, :], stages[mi][:])

            do_group(0, n_outer=True)     # w arrives n-by-n
            for g in range(1, NXG):
                do_group(g, n_outer=False)  # mi-outer spreads the writes

    nc.compile()
    return nc


def make_in_maps(input_, weight):
    X = np.asarray(input_, dtype=np.float32).reshape(M, H)
    W = np.asarray(weight, dtype=np.float32)
    in_maps = []
    for c in range(N_CORES):
        i, j = divmod(c, G_COL)
        # xH[g, p, k, mg] = X[i*M_LOC + g*XG + mg, k*P + p]
        xc = X[i * M_LOC:(i + 1) * M_LOC]                  # [M_LOC, H]
        xh = np.ascontiguousarray(
            xc.reshape(NXG, XG, KO, P).transpose(0, 3, 2, 1)
        )
        # wH[n, p, k, nq] = W[j*N_LOC + n*NT + nq, k*P + p]
        wc = W[j * N_LOC:(j + 1) * N_LOC]                  # [N_LOC, H]
        wh = np.ascontiguousarray(
            wc.reshape(NO, NT, KO, P).transpose(0, 3, 2, 1)
        )
        in_maps.append({"xH": xh, "wH": wh})
    return in_maps


def assemble(results):
    Y = np.empty((M, OUT), dtype=np.float32)
    for c in range(N_CORES):
        i, j = divmod(c, G_COL)
        Y[i * M_LOC:(i + 1) * M_LOC, j * N_LOC:(j + 1) * N_LOC] = results[c]["y"]
    return Y.reshape(S, B, OUT)


def kernel(input_, weight):
    nc = build_nc()
    res = run_bass_kernel_spmd(nc, make_in_maps(input_, weight), list(range(N_CORES)))
    return assemble(res.results)


# revision 12
# speedup vs baseline: 1.1572x; 1.0401x over previous
"""ColumnParallelLinear kernel for Trainium2 (8 NeuronCores).

Computes Y[s,b,o] = sum_h X[s,b,h] * W[o,h]  (F.linear / einsum 'sbh,oh->sbo')
with S,B,H,OUT = 2048,4,1024,4096, fp32.

Strategy:
  - Flatten tokens: M = S*B = 8192 rows.  GEMM: [M,H] @ [H,OUT].
  - 2D shard over 8 cores: 4 token groups (2048 rows) x 2 out-column
    groups (2048 cols).  This minimizes per-core HBM traffic
    (x 8.4MB + w 8.4MB + y 16.8MB = 33.6MB/core) vs pure column- or
    row-parallel, keeping the kernel compute-bound.
  - Host packs X and W into [chunk][partition][k][free] layout so every
    DMA descriptor moves a 16KB contiguous run (DMA is descriptor-rate
    limited at small runs); y is written as full 8KB rows.
  - Matmuls run as float32r (fp32 bits, full-rate PE path; moving dim
    512), accumulating fp32 in PSUM.
"""

import numpy as np

import concourse.bass as bass
from concourse import bacc
import concourse.mybir as mybir
import concourse.tile as tile
from concourse.bass_utils import run_bass_kernel_spmd

S, B, H, OUT = 2048, 4, 1024, 4096
M = S * B

N_CORES = 8
G_ROW, G_COL = 4, 2          # token groups x out-feature groups
M_LOC = M // G_ROW           # 2048 rows per core
N_LOC = OUT // G_COL         # 2048 out features per core

P = 128
KO = H // P                  # 8 contraction subtiles
NT = 512                     # psum free dim (one fp32 bank)
NO = N_LOC // NT             # 4 col tiles
XG = 512                     # x chunk width (4 row tiles)
NXG = M_LOC // XG            # 4 chunks
MO = M_LOC // P              # 16 row tiles

MM_DT = mybir.dt.float32r    # full-rate fp32 matmul path


def build_nc(mm_dt=MM_DT):
    nc = bacc.Bacc(None, target_bir_lowering=False)
    # packed inputs: [chunk][partition p][k][free] so each partition's slice
    # of one chunk is 16KB contiguous in DRAM (one descriptor per partition)
    xH = nc.declare_dram_parameter("xH", [NXG, P, KO, XG], mybir.dt.float32,
                                   isOutput=False)
    wH = nc.declare_dram_parameter("wH", [NO, P, KO, NT], mybir.dt.float32,
                                   isOutput=False)
    y = nc.declare_dram_parameter("y", [M_LOC, N_LOC], mybir.dt.float32,
                                  isOutput=True)
    y_r = y[:, :].rearrange("(mo p) n -> p mo n", p=P)

    with tile.TileContext(nc) as tc:
        with (
            tc.tile_pool(name="xp", bufs=1) as xp,
            tc.tile_pool(name="wp", bufs=1) as wp,
            tc.tile_pool(name="op", bufs=2) as op,
            tc.tile_pool(name="psp", bufs=8, space="PSUM") as psp,
        ):
            x_sb = [None] * NXG
            w_sb = [None] * NO
            KQ = 2  # k-pair granularity for the startup-critical chunks

            def load_x(g, split=False):
                if split:
                    # k-paired sub-tiles: the PE starts accumulating as soon
                    # as the first k-pair lands (packed layout keeps each
                    # sub-job's per-partition run contiguous)
                    subs = []
                    for q in range(KO // KQ):
                        t = xp.tile([P, KQ, XG], mm_dt, tag=f"x{g}q{q}",
                                    name=f"x{g}q{q}")
                        nc.sync.dma_start(
                            t[:], xH[g, :, q * KQ:(q + 1) * KQ, :].bitcast(mm_dt)
                        )
                        subs.append(t)
                    x_sb[g] = ("split", subs)
                else:
                    t = xp.tile([P, KO, XG], mm_dt, tag=f"x{g}", name=f"x{g}")
                    nc.sync.dma_start(t[:], xH[g, :, :, :].bitcast(mm_dt))
                    x_sb[g] = ("whole", t)

            def load_w(n, split=False):
                if split:
                    subs = []
                    for q in range(KO // KQ):
                        t = wp.tile([P, KQ, NT], mm_dt, tag=f"w{n}q{q}",
                                    name=f"w{n}q{q}")
                        nc.sync.dma_start(
                            t[:], wH[n, :, q * KQ:(q + 1) * KQ, :].bitcast(mm_dt)
                        )
                        subs.append(t)
                    w_sb[n] = ("split", subs)
                else:
                    t = wp.tile([P, KO, NT], mm_dt, tag=f"w{n}", name=f"w{n}")
                    nc.sync.dma_start(t[:], wH[n, :, :, :].bitcast(mm_dt))
                    w_sb[n] = ("whole", t)

            def slice_k(entry, k, lo, hi):
                kind, t = entry
                if kind == "whole":
                    return t[:, k, lo:hi]
                return t[k // KQ][:, k % KQ, lo:hi]

            # arrival order matches consumption: g0 runs n-outer so it only
            # needs w_n just-in-time while the rest streams in; the first
            # chunk pair is k-split so the PE ramps during arrival
            load_w(0, split=True)
            load_x(0, split=True)
            load_w(1)
            load_w(2)
            load_w(3)
            load_x(1)
            load_x(2)
            load_x(3)

            def do_group(g, n_outer, tail=False):
                stages = [op.tile([P, N_LOC], mybir.dt.float32, tag=f"st{mi}",
                                  name=f"st{g}_{mi}")
                          for mi in range(XG // P)]
                outer = range(NO) if n_outer else range(XG // P)
                inner = range(XG // P) if n_outer else range(NO)
                for a in outer:
                    for b in inner:
                        n, mi = (a, b) if n_outer else (b, a)
                        ps = psp.tile([P, NT], mybir.dt.float32)
                        for k in range(KO):
                            nc.tensor.matmul(
                                ps[:],
                                lhsT=slice_k(x_sb[g], k, mi * P, (mi + 1) * P),
                                rhs=slice_k(w_sb[n], k, 0, NT),
                                start=(k == 0),
                                stop=(k == KO - 1),
                            )
                        nc.vector.tensor_copy(
                            stages[mi][:, n * NT:(n + 1) * NT], ps[:]
                        )
                        if tail and mi == XG // P - 1:
                            # final stage: per-n writes right after each copy
                            # so the last write trails the last matmul by as
                            # little as possible
                            mo = g * (XG // P) + mi
                            nc.scalar.dma_start(
                                y_r[:, mo, n * NT:(n + 1) * NT],
                                stages[mi][:, n * NT:(n + 1) * NT],
                            )
                # full 8KB-run row writes on the ACT HWDGE ring
                last = XG // P - (1 if tail else 0)
                for mi in range(last):
                    mo = g * (XG // P) + mi
                    nc.scalar.dma_start(y_r[:, mo, :], stages[mi][:])

            do_group(0, n_outer=True)     # w arrives n-by-n
            for g in range(1, NXG):
                # mi-outer spreads the writes
                do_group(g, n_outer=False, tail=(g == NXG - 1))

    nc.compile()
    return nc


def make_in_maps(input_, weight):
    X = np.asarray(input_, dtype=np.float32).reshape(M, H)
    W = np.asarray(weight, dtype=np.float32)
    in_maps = []
    for c in range(N_CORES):
        i, j = divmod(c, G_COL)
        # xH[g, p, k, mg] = X[i*M_LOC + g*XG + mg, k*P + p]
        xc = X[i * M_LOC:(i + 1) * M_LOC]                  # [M_LOC, H]
        xh = np.ascontiguousarray(
            xc.reshape(NXG, XG, KO, P).transpose(0, 3, 2, 1)
        )
        # wH[n, p, k, nq] = W[j*N_LOC + n*NT + nq, k*P + p]
        wc = W[j * N_LOC:(j + 1) * N_LOC]                  # [N_LOC, H]
        wh = np.ascontiguousarray(
            wc.reshape(NO, NT, KO, P).transpose(0, 3, 2, 1)
        )
        in_maps.append({"xH": xh, "wH": wh})
    return in_maps


def assemble(results):
    Y = np.empty((M, OUT), dtype=np.float32)
    for c in range(N_CORES):
        i, j = divmod(c, G_COL)
        Y[i * M_LOC:(i + 1) * M_LOC, j * N_LOC:(j + 1) * N_LOC] = results[c]["y"]
    return Y.reshape(S, B, OUT)


def kernel(input_, weight):
    nc = build_nc()
    res = run_bass_kernel_spmd(nc, make_in_maps(input_, weight), list(range(N_CORES)))
    return assemble(res.results)
